# revision 47
# baseline (speedup 1.0000x reference)
"""Trainium2 Bass kernel for AudioGRUModel: GRU over 256 steps, final hidden.

Strategy: 8-way data-parallel over batch (32 rows/core), weights replicated.
All on-chip layouts are transposed ([feature-dim on partitions, batch on
free]) so the sequential recurrence needs no per-step transposes.

905us -> 684us in this session. The steady-state period (~2.54us/step) is a
fully-serial loop: sigma_z(367) -> zh(225) -> [64 hh matmuls @25ns: zh-half,
u-half, n-bank, z-bank] -> drain(175) + sem(90) -> sigma_z, co-binding with
the u-path (u -> u-half MMs -> sigma_r -> tt -> vv -> tanh -> u). Main
optimizations, in measured-impact order:

* post-compile _strip_sem_updates (-135us): the PE posts sem updates at
  ~34ns each while MMs complete every 25ns, so after a 32-MM burst the
  counter lags ~350ns, delaying every cross-engine hand-off on the chain.
  Stripping the ~94% of increments no wait targets makes the remaining
  ones post at true completion. Wait values are exactly remapped; sem id
  is found dynamically (it shifts when instructions are added).
* u = (1-z)*tanh(vv) computed as ONE STT (-46us): tanh is emitted with
  scale=-1 (odd function) so nn = -tanh(vv), then u = (z-1)*nn. This
  removed zc and its sigma_z->zc->u serialization from the chain.
* selector STARTs (r/n/z identity+bhhn matmuls, no data deps) hoisted into
  the PE idle window before the hh stream (-28us).
* whh stored fp8 e4m3 scaled by WSC=16 (out of subnormal range), moving
  operands stay bf16; compensated for free via sigmoid scale=1/16 and a
  1/16 in the tt STT. rel err 8.3e-3 -> 1.24e-2, still < the 2e-2 gate.
  (LDW-pair rate stayed 25ns — the win is DMA bytes + margin for later.)
* startup (-8us): ACT tables preloaded via dummy activations; weight DMAs
  split across sync/scalar/gpsimd queues in two waves (critical wih +
  first half-slab first); ~46 dummy matmuls warm the HAM clock gate
  (1.2->2.4GHz) during the DMA wait; single strided output DMA.
* gi (input projection) lives in a 16-step SBUF window (bf16, now 16x
  scaled); selector matmuls accumulate it into PSUM with the hh stream.
* double-sourced r-bank: next step's r matmuls consume zh and u as two
  moving operands (PSUM accumulates), so the r stream starts mid-chain.
* explicit engine-FIFO order chains on PE/ACT/DVE pin the schedule.
* x host-rearranged to [INP, slab, SQ, BL] (step-major): slab DMAs are
  contiguous AND land directly in matmul layout (no on-chip transpose).

Known dead ends (measured): GpSimd tensor ops are 429ns + ~150ns dispatch
(too slow for anything near the chain); splitting chain ops into column
halves adds DVE/ACT slot overhead that eats the latency win; PSUM has no
free banks for activation outputs; fp8 does not speed LDWEIGHTS issue.
"""

import numpy as np
import ml_dtypes

import concourse.bass as bass
import concourse.tile as tile
from concourse import mybir, bacc
from concourse.tile import add_dep_helper
from concourse.bass_utils import run_bass_kernel_spmd

F32 = mybir.dt.float32
BF16 = mybir.dt.bfloat16
F8E4 = mybir.dt.float8e4
AF = mybir.ActivationFunctionType
WSC = 16.0            # whh is stored fp8(e4m3) scaled by WSC (keeps values
                      # out of e4m3's subnormal range); PSUM accumulates
                      # 16x gates, compensated by sigmoid scale=1/WSC and a
                      # 1/WSC factor folded into the tt multiply

B, INP, S, H = 256, 512, 256, 512
G3 = 3 * H            # 1536
NC = 8
BL = B // NC          # 32 batch rows per core
KC = H // 128         # 4 contraction chunks
MC = G3 // 128        # 12 output chunks (0-3 r, 4-7 z, 8-11 n)
SQ = 64               # steps per x-staging slab
SG = 16               # steps per 512-col projection group
LEAD = 1              # projection groups kept ahead of the recurrence


def _dedup_ldweights(nc):
    """Remove LDWEIGHTS that reload the exact weights already resident."""
    removed = 0
    for f in nc.m.functions:
        for bb in f.blocks:
            insts = bb.instructions
            del_ids = set()
            last_key = None
            for i in insts:
                if type(i).__name__ == 'InstLdweights':
                    a = i.ins[0]
                    k = (a.memref, a.offset, str(a.ap), str(a.dtype),
                         str(i.perf_mode), str(i.tile_position))
                    has_sync = bool(i.sync_info and
                                    (i.sync_info.on_wait or i.sync_info.on_update))
                    if k == last_key and not has_sync:
                        del_ids.add(id(i))
                        continue
                    last_key = k
            if del_ids:
                insts[:] = [i for i in insts if id(i) not in del_ids]
            removed += len(del_ids)
    return removed


def _pe_sem_id(nc):
    """The semaphore the matmul stream increments (id shifts with edits)."""
    from collections import Counter
    c = Counter()
    for f in nc.m.functions:
        for bb in f.blocks:
            for i in bb.instructions:
                if type(i).__name__ == 'InstMatmult' and i.sync_info:
                    for x in (i.sync_info.on_update or []):
                        c[x.id] += 1
    return c.most_common(1)[0][0]


def _strip_sem_updates(nc, sem_id):
    """Keep only the PE sem-164 increments some wait actually targets.

    The PE posts semaphore updates at ~34ns apiece while matmuls complete
    every ~25ns, so during a 32-MM burst the counter falls ~300-400ns
    behind completion — observed directly delaying the σr/tt critical
    chain every step. Stripping the 94% of increments nothing waits on
    lets the remaining ones post immediately.
    """
    insts = []
    for f in nc.m.functions:
        for bb in f.blocks:
            insts.extend(bb.instructions)
    needed = set()
    for i in insts:
        si = i.sync_info
        if si:
            for x in (si.on_wait or []):
                if x.id == sem_id:
                    needed.add(x.wait_value)
    kept_counts = []
    c = 0
    for i in insts:
        si = i.sync_info
        if not si:
            continue
        upds = si.on_update or []
        hit = False
        for x in upds:
            if x.id == sem_id:
                assert x.update_mode == 'sem-inc' and x.update_value == 1
                c += 1
                hit = c in needed
                if hit:
                    kept_counts.append(c)
        if upds and any(x.id == sem_id for x in upds) and not hit:
            si.on_update = [x for x in upds if x.id != sem_id]
    import bisect
    for i in insts:
        si = i.sync_info
        if si:
            for x in (si.on_wait or []):
                if x.id == sem_id:
                    x.wait_value = bisect.bisect_right(kept_counts, x.wait_value)
    return c, len(kept_counts)


def _build(steps=S):
    nc = bacc.Bacc("TRN2", target_bir_lowering=False, debug=False)

    # x arrives host-rearranged to [INP, n_slabs, SQ, BL] (step-major) so
    # each slab DMA reads 4KB-contiguous runs per partition AND lands
    # directly in the [s, b] layout the projection matmuls consume — no
    # on-chip transpose needed.
    nslab = (steps + SQ - 1) // SQ
    xb_d = nc.dram_tensor("x_bf", [INP, nslab, SQ, BL], BF16,
                          kind="ExternalInput")
    wih_d = nc.dram_tensor("wih_t", [INP, G3], BF16, kind="ExternalInput")
    whh_d = nc.dram_tensor("whh_t", [H, G3], F8E4, kind="ExternalInput")
    # bhhn/sel32 padded to K=128: a K=4 stationary would be a partial
    # row-group LDWEIGHTS, which stalls the PE pipeline mid-stream
    bsum_d = nc.dram_tensor("bsum", [128, MC], F32, kind="ExternalInput")
    bhhn_d = nc.dram_tensor("bhhn", [128, 128], BF16, kind="ExternalInput")
    sel_d = nc.dram_tensor("sel32", [128, 128], BF16, kind="ExternalInput")
    id_d = nc.dram_tensor("ident", [128, 128], BF16, kind="ExternalInput")
    # output stays transposed ([H, BL]) so the final DMA is contiguous;
    # the host transposes (a [b p -> p b] scatter DMA here cost ~75us)
    out_d = nc.dram_tensor("h_out", [H, BL], F32, kind="ExternalOutput")

    all_mms = []

    def mm(*args, **kwargs):
        m = nc.tensor.matmul(*args, **kwargs)
        if all_mms:
            add_dep_helper(m.ins, all_mms[-1].ins, False, "pe-order")
        all_mms.append(m)
        return m

    # Force engine-FIFO order to match emission order on ACT and DVE too —
    # the Tile scheduler otherwise interleaves projection evacuations into
    # the serial gate chain (measured: tanh stalled ~850ns behind an evac).
    last_act = []
    last_dve = []

    def act(fn, *args, **kwargs):
        i = fn(*args, **kwargs)
        if last_act:
            add_dep_helper(i.ins, last_act[0].ins, False, "act-order")
        last_act[:] = [i]
        return i

    def dve(fn, *args, **kwargs):
        i = fn(*args, **kwargs)
        if last_dve:
            add_dep_helper(i.ins, last_dve[0].ins, False, "dve-order")
        last_dve[:] = [i]
        return i

    ngroups = steps // SG

    with tile.TileContext(nc) as tc:
        with (
            tc.tile_pool(name="consts", bufs=1) as consts,
            tc.tile_pool(name="xstage", bufs=2) as xstage,
            tc.tile_pool(name="win", bufs=3) as winp,
            tc.tile_pool(name="ipsum", bufs=2, space="PSUM") as ipsum,
            tc.tile_pool(name="pr", bufs=2, space="PSUM") as prp,
            tc.tile_pool(name="pz", bufs=2, space="PSUM") as pzp,
            tc.tile_pool(name="pn", bufs=2, space="PSUM") as pnp,
            tc.tile_pool(name="gates", bufs=2) as gates,
        ):
            # ---- constants / weights ----
            # DMA issue order matters at startup (the Sync queue issues them
            # serially): slab 0 + wih first (they gate the first projection),
            # whh last (first needed by step 1's matmuls, ~25us in)
            wih = consts.tile([128, KC, G3], BF16)
            whh = consts.tile([128, KC, G3], F8E4)
            bsum = consts.tile([128, MC], F32)
            nc.sync.dma_start(out=bsum[:], in_=bsum_d.ap())
            bhhn = consts.tile([128, 128], BF16)
            nc.sync.dma_start(out=bhhn[:], in_=bhhn_d.ap())
            sel32 = consts.tile([128, 128], BF16)
            nc.sync.dma_start(out=sel32[:], in_=sel_d.ap())
            ident = consts.tile([128, 128], BF16)
            nc.sync.dma_start(out=ident[:], in_=id_d.ap())
            ones = consts.tile([128, 128], BF16)
            nc.vector.memset(ones[:], 1.0)
            # pre-load ACT function tables (Sigmoid/Tanh/Identity) during the
            # DMA window — otherwise the sigmoid table load (~1.3us) lands
            # right before the first real sigmoid
            warm = consts.tile([128, 1], BF16, name="actwarm")
            nc.scalar.activation(warm[:], ones[:, 0:1], AF.Identity)
            nc.scalar.activation(warm[:], ones[:, 0:1], AF.Sigmoid)
            nc.scalar.activation(warm[:], ones[:, 0:1], AF.Tanh)

            # h state, bf16, ping-pong buffers
            hb = [consts.tile([128, 128], BF16, name=f"hb{i}") for i in range(2)]
            nc.vector.memset(hb[0][:], 0.0)
            nc.vector.memset(hb[1][:], 0.0)

            # ---- input-projection machinery (emitted incrementally) ----
            # gi window tiles: [128, MC, SG, BL] bf16, one per 16-step group
            slab_tiles = {}
            win_tiles = {}

            def stage_slab(q, halves=False):
                xt3 = xstage.tile([128, KC, SQ, BL], BF16, name="xt3", tag="xt3")
                if halves:
                    # first half (steps 0..31 of the slab) lands first so the
                    # LEAD projections can start ~7us earlier at kernel start
                    hq = SQ // 2
                    for k in range(KC):
                        nc.sync.dma_start(
                            out=xt3[:, k, 0:hq, :],
                            in_=xb_d[128 * k:128 * (k + 1), q, 0:hq, :],
                        )
                    for k in range(KC):
                        nc.sync.dma_start(
                            out=xt3[:, k, hq:SQ, :],
                            in_=xb_d[128 * k:128 * (k + 1), q, hq:SQ, :],
                        )
                else:
                    for k in range(KC):
                        nc.sync.dma_start(
                            out=xt3[:, k, :, :],
                            in_=xb_d[128 * k:128 * (k + 1), q, :, :],
                        )
                slab_tiles[q] = xt3

            ip_state = {}

            def iproj_mm(g, j):
                """Emit the j-th projection matmul (of 48) for step-group g."""
                m_, k = j // KC, j % KC
                xt3 = slab_tiles[g // (SQ // SG)]
                goff = (g % (SQ // SG)) * SG
                if j == 0:
                    win_tiles[g] = winp.tile([128, MC, SG, BL], BF16,
                                             name="win", tag="win")
                if k == 0:
                    ip_state[g] = ipsum.tile([128, SG * BL], F32,
                                             name="ips", tag="ips")
                ps = ip_state[g]
                mm(ps[:], wih[:, k, 128 * m_:128 * (m_ + 1)],
                   xt3[:, k, goff:goff + SG, :],
                   start=(k == 0), stop=(k == KC - 1))
                if k == KC - 1:
                    # evacuate with bias straight into the bf16 SBUF window
                    act(nc.scalar.activation,
                        win_tiles[g][:, m_, :, :], ps[:], AF.Identity,
                        bias=bsum[:, m_:m_ + 1], scale=1.0)

            # up-front DMA, two waves on three queues (sync/scalar/gpsimd).
            # Wave 1 is only what the LEAD projections need (wih + first half
            # of slab 0, ~2.5MB); whh + the rest follow — HBM BW is the
            # startup floor, so non-critical bytes must not steal it.
            xt3_0 = xstage.tile([128, KC, SQ, BL], BF16, name="xt3", tag="xt3")
            slab_tiles[0] = xt3_0
            hq = SQ // 2
            for k in range(KC):
                nc.sync.dma_start(out=xt3_0[:, k, 0:hq, :],
                                  in_=xb_d[128 * k:128 * (k + 1), 0, 0:hq, :])
            for k in range(2):
                nc.scalar.dma_start(out=wih[:, k, :],
                                    in_=wih_d[128 * k:128 * (k + 1), :])
            for k in range(2, KC):
                nc.gpsimd.dma_start(out=wih[:, k, :],
                                    in_=wih_d[128 * k:128 * (k + 1), :])
            # wave 2
            for k in range(KC):
                nc.sync.dma_start(out=xt3_0[:, k, hq:SQ, :],
                                  in_=xb_d[128 * k:128 * (k + 1), 0, hq:SQ, :])
            for k in range(2):
                nc.scalar.dma_start(out=whh[:, k, :],
                                    in_=whh_d[128 * k:128 * (k + 1), :])
            for k in range(2, KC):
                nc.gpsimd.dma_start(out=whh[:, k, :],
                                    in_=whh_d[128 * k:128 * (k + 1), :])
            # HAM warm-up: dummy matmuls on scratch data while the DMAs run,
            # so the LEAD projections (and first steps) run at 2.4GHz not 1.2
            scratch = consts.tile([128, 512], BF16, name="scratch")
            nc.vector.memset(scratch[:], 0.0)
            for i in range(46):
                wps = ipsum.tile([128, SG * BL], F32, name="ips", tag="ips")
                mm(wps[:], ones[:], scratch[:], start=True, stop=True)
            up = min(LEAD, ngroups)
            for g in range(up):
                for m_ in range(MC):
                    for k in range(KC):
                        iproj_mm(g, m_ * KC + k)

            # ---- recurrence with interleaved projection ----
            for t in range(steps):
                # stage slab q a full slab-window ahead of its first use
                for q in range(1, (steps + SQ - 1) // SQ):
                    if t == SQ * (q - 1):
                        stage_slab(q)

                win = win_tiles[t // SG]
                toff = t % SG
                h_in = hb[t % 2]
                h_out = hb[(t + 1) % 2]

                # --- PE: r bank first. Instead of waiting for h = u + zh, the
                # r matmuls consume zh and u as separate moving operands (PSUM
                # accumulates), so the zh half streams during the chain tail
                # and the u half fires straight off the u multiply.
                # all three selector STARTs go first: they have no data deps
                # (win/bhhn only) and run in the PE idle window while zh is
                # still being computed — keeps them out of the critical
                # zh-half -> u-half -> n -> z matmul stream
                p_r = prp.tile([128, 128], F32, name="pr", tag="pr")
                mm(p_r[:], ident[:], win[:, 0:4, toff, :],
                   start=True, stop=(t == 0))
                p_n = pnp.tile([128, 128], F32, name="pn", tag="pn")
                mm(p_n[:], bhhn[:], sel32[:], start=True, stop=(t == 0))
                p_z = pzp.tile([128, 128], F32, name="pz", tag="pz")
                mm(p_z[:], ident[:], win[:, 4:8, toff, :],
                   start=True, stop=(t == 0))
                if t > 0:
                    for src in (zh_prev, u_prev):
                        for m_ in range(4):
                            for k in range(KC):
                                mm(p_r[:, 32 * m_:32 * (m_ + 1)],
                                   whh[:, k, 128 * m_:128 * (m_ + 1)],
                                   src[:, 32 * k:32 * (k + 1)],
                                   start=False,
                                   stop=(src is u_prev) and (m_ == 3)
                                   and (k == KC - 1))
                    # n bank next, so tt = r*p_n isn't starved
                    for m_ in range(8, MC):
                        c0 = 32 * (m_ - 8)
                        for k in range(KC):
                            mm(p_n[:, c0:c0 + 32],
                               whh[:, k, 128 * m_:128 * (m_ + 1)],
                               h_in[:, 32 * k:32 * (k + 1)],
                               start=False,
                               stop=(m_ == MC - 1) and (k == KC - 1))
                    # z matmuls last (z is only needed late, for zh)
                    for m_ in range(4, 8):
                        for k in range(KC):
                            mm(p_z[:, 32 * (m_ - 4):32 * (m_ - 3)],
                               whh[:, k, 128 * m_:128 * (m_ + 1)],
                               h_in[:, 32 * k:32 * (k + 1)],
                               start=False,
                               stop=(m_ == 7) and (k == KC - 1))

                # --- gate chain (ACT: sig_r, sig_z, tanh; DVE: the rest) ---
                r = gates.tile([128, 128], BF16, name="r", tag="r")
                act(nc.scalar.activation, r[:], p_r[:], AF.Sigmoid,
                    scale=1.0 / WSC)
                z = gates.tile([128, 128], BF16, name="z", tag="z")
                act(nc.scalar.activation, z[:], p_z[:], AF.Sigmoid,
                    scale=1.0 / WSC)

                tt = gates.tile([128, 128], BF16, name="tt", tag="tt")
                dve(nc.vector.scalar_tensor_tensor, tt[:], p_n[:], 1.0 / WSC,
                    r[:], mybir.AluOpType.mult, mybir.AluOpType.mult)
                vv = gates.tile([128, 128], BF16, name="vv", tag="vv")
                dve(nc.vector.tensor_add, vv[:], tt[:], win[:, 8:12, toff, :])
                # nn = -tanh(vv) via scale=-1 (tanh is odd), so that
                # u = (1-z)*tanh(vv) = (z-1)*nn is a single STT — this
                # removes zc (and its ~600ns sigma_z->zc->u serialization)
                # from the chain entirely
                nn = gates.tile([128, 128], BF16, name="nn", tag="nn")
                act(nc.scalar.activation, nn[:], vv[:], AF.Tanh, scale=-1.0)

                zh = gates.tile([128, 128], BF16, name="zh", tag="zh")
                dve(nc.vector.tensor_mul, zh[:], z[:], h_in[:])
                u = gates.tile([128, 128], BF16, name="u", tag="u")
                dve(nc.vector.scalar_tensor_tensor, u[:], z[:], 1.0, nn[:],
                    mybir.AluOpType.subtract, mybir.AluOpType.mult)
                dve(nc.vector.tensor_add, h_out[:], u[:], zh[:])
                u_prev, zh_prev = u, zh

                # --- off-path work: projection matmuls + evac for
                # group t//SG + LEAD
                g = t // SG + LEAD
                if g < ngroups:
                    j0 = 3 * toff
                    for j in (j0, j0 + 1, j0 + 2):
                        iproj_mm(g, j)

            # ---- output: cast to fp32 and un-transpose h^T -> h ----
            hf = consts.tile([128, 128], F32, name="hf")
            dve(nc.vector.tensor_copy, hf[:], hb[steps % 2][:])
            nc.sync.dma_start(
                out=out_d.ap().rearrange("(k p) b -> p k b", k=KC),
                in_=hf[:].rearrange("p (k b) -> p k b", k=KC),
            )

    nc.compile()
    _dedup_ldweights(nc)
    _strip_sem_updates(nc, _pe_sem_id(nc))
    return nc


def _prep_inputs(x, weight_ih, weight_hh, bias_ih, bias_hh):
    x = np.ascontiguousarray(np.asarray(x, dtype=np.float32))
    w_ih = np.asarray(weight_ih, dtype=np.float32)
    w_hh = np.asarray(weight_hh, dtype=np.float32)
    b_ih = np.asarray(bias_ih, dtype=np.float32)
    b_hh = np.asarray(bias_hh, dtype=np.float32)

    x_bf = x.astype(ml_dtypes.bfloat16)
    wih_t = np.ascontiguousarray(w_ih.T).astype(ml_dtypes.bfloat16)
    # whh stored fp8 e4m3, pre-scaled by WSC=16 (see kernel.py header)
    whh_t = (np.ascontiguousarray(w_hh.T) * 16.0).astype(ml_dtypes.float8_e4m3)
    bsum = np.empty((128, MC), np.float32)
    for m in range(MC):
        seg = b_ih[128 * m:128 * (m + 1)].copy()
        if m < 8:
            seg += b_hh[128 * m:128 * (m + 1)]
        bsum[:, m] = seg
    # bhhn and ident are PSUM-bound stationaries -> carry the 16x scale
    bhhn = np.zeros((128, 128), np.float32)
    bhhn[:KC] = 16.0 * b_hh[2 * H:].reshape(KC, 128)
    bhhn = bhhn.astype(ml_dtypes.bfloat16)
    sel32 = np.zeros((128, 128), np.float32)
    for k in range(KC):
        sel32[k, 32 * k:32 * (k + 1)] = 1.0
    sel32 = sel32.astype(ml_dtypes.bfloat16)
    ident = (16.0 * np.eye(128, dtype=np.float32)).astype(ml_dtypes.bfloat16)

    shared = {"wih_t": wih_t, "whh_t": whh_t, "bsum": bsum,
              "bhhn": bhhn, "sel32": sel32, "ident": ident}
    in_maps = []
    for c in range(NC):
        m = dict(shared)
        xc = x_bf[BL * c:BL * (c + 1)].transpose(1, 0, 2)      # [INP, BL, S]
        xc = xc.reshape(INP, BL, S // SQ, SQ).transpose(0, 2, 3, 1)
        m["x_bf"] = np.ascontiguousarray(xc)                   # [INP, q, SQ, BL]
        in_maps.append(m)
    return in_maps


_NC_CACHE = {}


def _get_nc(steps=S):
    if steps not in _NC_CACHE:
        _NC_CACHE[steps] = _build(steps)
    return _NC_CACHE[steps]


def kernel(x, weight_ih, weight_hh, bias_ih, bias_hh):
    nc = _get_nc(S)
    in_maps = _prep_inputs(x, weight_ih, weight_hh, bias_ih, bias_hh)
    res = run_bass_kernel_spmd(nc, in_maps, core_ids=list(range(NC)))
    return np.concatenate(
        [np.asarray(res.results[c]["h_out"]).T for c in range(NC)], axis=0
    ).astype(np.float32)

